# revision 1
# baseline (speedup 1.0000x reference)
"""MoE routing layer on 8 Trainium2 NeuronCores (data-parallel over batch).

Per core (4 samples):
  routing MLP -> cosine sim vs embeddings -> softmax weights wf[4,10]
  w_eff[b] = sum_n wf[b,n] * conv_w[n]  (conv is linear in weights ->
  10x fewer conv FLOPs than materializing all expert convs)
  out[b] = conv2d(x[b], w_eff[b]) + b_eff[b]

Conv is 9 shifted matmuls over the flat 58-wide grid (tap = constant
free-dim offset); two samples run concurrently on the PE array via
row tiling (partitions 0-63 / 64-127), fp32r for full-rate streaming.
"""
import sys

sys.path.insert(0, "/opt/trn_rl_repo")

import numpy as np

import concourse.bass as bass
import concourse.mybir as mybir
from concourse.masks import make_identity
from concourse.tile import TileContext

F32 = mybir.dt.float32
F32R = mybir.dt.float32r
AF = mybir.ActivationFunctionType
ALU = mybir.AluOpType
AX = mybir.AxisListType

NCORES = 8
BLOC = 4           # samples per core
CIN = 64
COUT = 64
H = W = 58
HW = H * W         # 3364
OH = OW = 56
NB = 10            # experts
EDIM = 64
RSIZE = 512
HID = 128
NTAP = 9
CHUNK_ROWS = 8
NCHUNK = 7         # 7*8 = 56 output rows
NFREE = CHUNK_ROWS * W  # 464 <= 512 (one PSUM bank)
TAP_OFF = [dy * W + dx for dy in range(3) for dx in range(3)]
PAIRED = True      # 2-sample row-tiled PE packing
CONV_DT = F32R


def fix_sync_waits(nc, cap=2):
    """This walrus build allows at most `cap` sem waits per instruction.
    Splice same-engine NoOps carrying the excess waits right before any
    over-subscribed instruction (waits happen earlier => same semantics)."""
    uid = [0]
    for f in nc.m.functions:
        for blk in f.blocks:
            insts = blk.instructions  # live list
            i = 0
            while i < len(insts):
                inst = insts[i]
                si = inst.sync_info
                waits = list(si.on_wait) if si and si.on_wait else []
                icap = 1
                if len(waits) <= icap:
                    i += 1
                    continue
                keep, excess = waits[-icap:], waits[:-icap]
                for k in range(0, len(excess), icap):
                    nop = mybir.InstNoOp(
                        name=f"{inst.name}-wsplit{uid[0]}", ins=[], outs=[]
                    )
                    uid[0] += 1
                    nop.engine = inst.engine
                    nop.sync_info = mybir.SyncInfo(
                        on_wait=excess[k : k + icap], on_update=[]
                    )
                    nc.register_instruction(nop, overwrite=True)
                    insts.insert(i, nop)
                    i += 1
                inst.sync_info = mybir.SyncInfo(
                    on_wait=keep,
                    on_update=list(si.on_update) if si and si.on_update else [],
                )
                i += 1


def build():
    nc = bass.Bass(num_swdge_queues=4)
    x = nc.dram_tensor("x", [BLOC, CIN, H, W], F32, kind="ExternalInput")
    rv = nc.dram_tensor("rv", [BLOC, RSIZE], F32, kind="ExternalInput")
    w1 = nc.dram_tensor("w1", [RSIZE, HID], F32, kind="ExternalInput")
    bias1 = nc.dram_tensor("bias1", [HID, 1], F32, kind="ExternalInput")
    w2 = nc.dram_tensor("w2", [HID, EDIM], F32, kind="ExternalInput")
    bias2 = nc.dram_tensor("bias2", [EDIM, 1], F32, kind="ExternalInput")
    emb = nc.dram_tensor("emb", [NB, EDIM], F32, kind="ExternalInput")
    cwp = nc.dram_tensor("cwp", [CIN, NB, NTAP, COUT], F32, kind="ExternalInput")
    cb = nc.dram_tensor("cb", [NB, COUT], F32, kind="ExternalInput")
    sel = nc.dram_tensor("sel", [2, BLOC, 128], F32, kind="ExternalInput")
    identin = nc.dram_tensor("identin", [128, 128], F32, kind="ExternalInput")
    out = nc.dram_tensor("out", [BLOC, COUT, OH, OW], F32, kind="ExternalOutput")

    with TileContext(nc) as tc:
        with (
            tc.tile_pool(name="consts", bufs=1) as consts,
            tc.tile_pool(name="work", bufs=2) as work,
            tc.tile_pool(name="stage", bufs=4) as stage,
            tc.tile_pool(name="ps", bufs=2, space="PSUM") as pspool,
            tc.tile_pool(name="psconv", bufs=2, space="PSUM") as psconv,
        ):
            # ---------- inputs / constants into SBUF ----------
            ident = consts.tile([128, 128], F32, tag="ident")
            nc.sync.dma_start(out=ident[:], in_=identin[:])
            ones64 = consts.tile([EDIM, 1], F32, tag="ones64")
            nc.vector.memset(ones64[:], 1.0)

            rvsb = consts.tile([BLOC, RSIZE], F32, tag="rvsb")
            nc.sync.dma_start(out=rvsb[:], in_=rv[:])
            w1sb = consts.tile([128, 4, HID], F32, tag="w1sb")
            nc.sync.dma_start(
                out=w1sb[:], in_=w1[:].rearrange("(c k) m -> k c m", k=128)
            )
            w2sb = consts.tile([HID, EDIM], F32, tag="w2sb")
            nc.sync.dma_start(out=w2sb[:], in_=w2[:])
            b1sb = consts.tile([HID, 1], F32, tag="b1sb")
            nc.sync.dma_start(out=b1sb[:], in_=bias1[:])
            b2sb = consts.tile([EDIM, 1], F32, tag="b2sb")
            nc.sync.dma_start(out=b2sb[:], in_=bias2[:])
            embsb = consts.tile([NB, EDIM], F32, tag="embsb")
            nc.sync.dma_start(out=embsb[:], in_=emb[:])
            cbsb = consts.tile([NB, COUT], F32, tag="cbsb")
            nc.sync.dma_start(out=cbsb[:], in_=cb[:])
            selsb = consts.tile([BLOC, 2, 128], F32, tag="selsb")
            nc.sync.dma_start(out=selsb[:], in_=sel[:].rearrange("j b p -> b j p"))

            cwp2 = consts.tile([128, NB, NTAP, COUT], F32, tag="cwp2")
            nc.sync.dma_start(out=cwp2[0:64], in_=cwp[:])
            nc.sync.dma_start(out=cwp2[64:128], in_=cwp[:])

            xt = []
            for j in range(2):
                t = consts.tile([128, HW + 4], CONV_DT, tag=f"xt{j}")
                nc.vector.memset(t[:, HW : HW + 4].bitcast(F32), 0.0)
                nc.gpsimd.dma_start(
                    out=t[0:64, 0:HW], in_=x[2 * j].rearrange("c h w -> c (h w)")
                )
                nc.gpsimd.dma_start(
                    out=t[64:128, 0:HW],
                    in_=x[2 * j + 1].rearrange("c h w -> c (h w)"),
                )
                xt.append(t)

            # ---------- routing MLP ----------
            # rv [4, 512] -> rvT [128, 4(chunk), 4(sample)] via PE transposes
            rvT = work.tile([128, 4, BLOC], F32, tag="rvT")
            for c in range(4):
                pst = pspool.tile([128, BLOC], F32, tag="small")
                nc.tensor.transpose(
                    pst[:], rvsb[:, c * 128 : (c + 1) * 128], ident[0:BLOC, 0:BLOC]
                )
                nc.scalar.copy(out=rvT[:, c, :], in_=pst[:])
            h1 = pspool.tile([HID, BLOC], F32, tag="small")
            for c in range(4):
                nc.tensor.matmul(
                    h1[:], w1sb[:, c, :], rvT[:, c, :], start=(c == 0), stop=(c == 3)
                )
            h1r = work.tile([HID, BLOC], F32, tag="h1r")
            nc.scalar.activation(
                out=h1r[:], in_=h1[:], func=AF.Relu, bias=b1sb[:], scale=1.0
            )
            rps = pspool.tile([EDIM, BLOC], F32, tag="small")
            nc.tensor.matmul(rps[:], w2sb[:], h1r[:], start=True, stop=True)
            rsb = work.tile([EDIM, BLOC], F32, tag="rsb")
            nc.scalar.activation(
                out=rsb[:], in_=rps[:], func=AF.Identity, bias=b2sb[:], scale=1.0
            )

            # ---------- cosine similarity ----------
            rsq = work.tile([EDIM, BLOC], F32, tag="rsq")
            nc.vector.tensor_mul(rsq[:], rsb[:], rsb[:])
            nsq = pspool.tile([BLOC, 1], F32, tag="small")
            nc.tensor.matmul(nsq[:], rsq[:], ones64[:], start=True, stop=True)
            rln = work.tile([BLOC, 1], F32, tag="rln")
            nc.scalar.activation(out=rln[:], in_=nsq[:], func=AF.Ln)
            rinv = work.tile([BLOC, 1], F32, tag="rinv")
            nc.scalar.activation(out=rinv[:], in_=rln[:], func=AF.Exp, scale=-0.5)

            esq = work.tile([NB, EDIM], F32, tag="esq")
            nc.vector.tensor_mul(esq[:], embsb[:], embsb[:])
            ensq = work.tile([NB, 1], F32, tag="ensq")
            nc.vector.tensor_reduce(ensq[:], esq[:], axis=AX.X, op=ALU.add)
            eln = work.tile([NB, 1], F32, tag="eln")
            nc.scalar.activation(out=eln[:], in_=ensq[:], func=AF.Ln)
            einv = work.tile([NB, 1], F32, tag="einv")
            nc.scalar.activation(out=einv[:], in_=eln[:], func=AF.Exp, scale=-0.5)
            embn = work.tile([NB, EDIM], F32, tag="embn")
            nc.vector.tensor_scalar_mul(out=embn[:], in0=embsb[:], scalar1=einv[:])
            embnT_ps = pspool.tile([EDIM, NB], F32, tag="small")
            nc.tensor.transpose(embnT_ps[:], embn[:], ident[0:NB, 0:NB])
            embnT = work.tile([EDIM, NB], F32, tag="embnT")
            nc.scalar.copy(out=embnT[:], in_=embnT_ps[:])

            simps = pspool.tile([BLOC, NB], F32, tag="small")
            nc.tensor.matmul(simps[:], rsb[:], embnT[:], start=True, stop=True)
            sim = work.tile([BLOC, NB], F32, tag="sim")
            nc.vector.tensor_scalar_mul(out=sim[:], in0=simps[:], scalar1=rinv[:])

            # ---------- softmax ----------
            mx = work.tile([BLOC, 1], F32, tag="mx")
            nc.vector.tensor_reduce(mx[:], sim[:], axis=AX.X, op=ALU.max)
            negmx = work.tile([BLOC, 1], F32, tag="negmx")
            nc.vector.tensor_scalar_mul(out=negmx[:], in0=mx[:], scalar1=-1.0)
            ex = work.tile([BLOC, NB], F32, tag="ex")
            nc.scalar.activation(
                out=ex[:], in_=sim[:], func=AF.Exp, bias=negmx[:], scale=1.0
            )
            s = work.tile([BLOC, 1], F32, tag="s")
            nc.vector.tensor_reduce(s[:], ex[:], axis=AX.X, op=ALU.add)
            sinv = work.tile([BLOC, 1], F32, tag="sinv")
            nc.vector.reciprocal(sinv[:], s[:])
            wf = work.tile([BLOC, NB], F32, tag="wf")
            nc.vector.tensor_scalar_mul(out=wf[:], in0=ex[:], scalar1=sinv[:])

            # ---------- effective conv bias ----------
            wfT_ps = pspool.tile([NB, BLOC], F32, tag="small")
            nc.tensor.transpose(wfT_ps[:], wf[:], ident[0:BLOC, 0:BLOC])
            wfT = work.tile([NB, BLOC], F32, tag="wfT")
            nc.scalar.copy(out=wfT[:], in_=wfT_ps[:])
            beff_ps = pspool.tile([COUT, BLOC], F32, tag="small")
            nc.tensor.matmul(beff_ps[:], cbsb[:], wfT[:], start=True, stop=True)
            beff = work.tile([COUT, BLOC], F32, tag="beff")
            nc.scalar.copy(out=beff[:], in_=beff_ps[:])

            # ---------- PE warmup: keep HAM busy until conv starts ----------
            warm_ps = pspool.tile([128, 512], F32, tag="warm")
            wl = ident[:].bitcast(mybir.dt.bfloat16)[:, 0:128]
            wr = w1sb[:].rearrange("p c m -> p (c m)").bitcast(mybir.dt.bfloat16)[:, 0:512]
            for _ in range(22):
                nc.tensor.matmul(warm_ps[:], wl, wr, start=True, stop=True)
            warm_sink = work.tile([1, 1], F32, tag="warm_sink")
            nc.scalar.copy(out=warm_sink[:], in_=warm_ps[0:1, 0:1])

            # ---------- both pairs: weights broadcast + w_eff first ----------
            weffs = []
            for j in range(2):
                wfbc_ps = pspool.tile([128, NB], F32, tag="small")
                nc.tensor.matmul(
                    wfbc_ps[:], selsb[:, j, :], wf[:], start=True, stop=True
                )
                wfbc = work.tile([128, NB], F32, tag=f"wfbc{j}")
                nc.scalar.copy(out=wfbc[:], in_=wfbc_ps[:])

                weff = work.tile([128, NTAP, COUT], CONV_DT, tag=f"weff{j}")
                for lo, hi in ((0, 5), (5, NTAP)):
                    nc.vector.tensor_scalar_mul(
                        out=weff[:, lo:hi], in0=cwp2[:, 0, lo:hi], scalar1=wfbc[:, 0:1]
                    )
                    for n in range(1, NB):
                        nc.vector.scalar_tensor_tensor(
                            out=weff[:, lo:hi],
                            in0=cwp2[:, n, lo:hi],
                            scalar=wfbc[:, n : n + 1],
                            in1=weff[:, lo:hi],
                            op0=ALU.mult,
                            op1=ALU.add,
                        )
                weffs.append(weff)

            # ---------- PE warmup: keep HAM busy until conv starts ----------
            warm_ps = pspool.tile([128, 512], F32, tag="warm")
            wl = ident[:].bitcast(mybir.dt.bfloat16)[:, 0:128]
            wr = w1sb[:].rearrange("p c m -> p (c m)").bitcast(mybir.dt.bfloat16)[:, 0:512]
            for _ in range(22):
                nc.tensor.matmul(warm_ps[:], wl, wr, start=True, stop=True)
            warm_sink = work.tile([1, 1], F32, tag="warm_sink")
            nc.scalar.copy(out=warm_sink[:], in_=warm_ps[0:1, 0:1])

            # ---------- conv ----------
            for j in range(2):
                weff = weffs[j]
                for ch in range(NCHUNK):
                    h0 = ch * CHUNK_ROWS
                    psA = psconv.tile([COUT, NFREE], F32, tag="psA")
                    psB = psconv.tile([COUT, NFREE], F32, tag="psB")
                    for t in range(NTAP):
                        off = h0 * W + TAP_OFF[t]
                        nc.tensor.matmul(
                            psA[:],
                            weff[0:64, t, :],
                            xt[j][0:64, off : off + NFREE],
                            start=(t == 0),
                            stop=(t == NTAP - 1),
                            tile_position=(0, 0) if PAIRED else None,
                        )
                        nc.tensor.matmul(
                            psB[:],
                            weff[64:128, t, :],
                            xt[j][64:128, off : off + NFREE],
                            start=(t == 0),
                            stop=(t == NTAP - 1),
                            tile_position=(64, 0) if PAIRED else None,
                        )
                    for half, ps in ((0, psA), (1, psB)):
                        b = 2 * j + half
                        st = stage.tile([COUT, CHUNK_ROWS, OW], F32, tag="st")
                        psv = ps[:].rearrange("p (r w) -> p r w", w=W)[:, :, 0:OW]
                        nc.scalar.activation(
                            out=st[:],
                            in_=psv,
                            func=AF.Identity,
                            bias=beff[:, b : b + 1],
                            scale=1.0,
                        )
                        nc.sync.dma_start(
                            out=out[b, :, h0 : h0 + CHUNK_ROWS, :], in_=st[:]
                        )

    fix_sync_waits(nc)
    return nc


_NC = None


def _get_nc():
    global _NC
    if _NC is None:
        _NC = build()
    return _NC


def make_in_maps(inputs):
    x = np.ascontiguousarray(np.asarray(inputs["x"], dtype=np.float32))
    rvec = np.ascontiguousarray(np.asarray(inputs["routing_vector"], dtype=np.float32))
    W1 = np.ascontiguousarray(np.asarray(inputs["W1"], dtype=np.float32))
    b1 = np.ascontiguousarray(np.asarray(inputs["b1"], dtype=np.float32)).reshape(HID, 1)
    W2 = np.ascontiguousarray(np.asarray(inputs["W2"], dtype=np.float32))
    b2 = np.ascontiguousarray(np.asarray(inputs["b2"], dtype=np.float32)).reshape(EDIM, 1)
    emb = np.ascontiguousarray(np.asarray(inputs["emb"], dtype=np.float32))
    conv_w = np.asarray(inputs["conv_w"], dtype=np.float32)
    conv_b = np.ascontiguousarray(np.asarray(inputs["conv_b"], dtype=np.float32))
    # conv_w[n, co, ci, ky, kx] -> cwp[ci, n, (ky kx), co]
    cwpa = np.ascontiguousarray(
        conv_w.transpose(2, 0, 3, 4, 1).reshape(CIN, NB, NTAP, COUT)
    )
    selm = np.zeros((2, BLOC, 128), np.float32)
    for j in range(2):
        selm[j, 2 * j, 0:64] = 1.0
        selm[j, 2 * j + 1, 64:128] = 1.0
    identm = np.eye(128, dtype=np.float32)
    in_maps = []
    for c in range(NCORES):
        in_maps.append(
            {
                "x": np.ascontiguousarray(x[BLOC * c : BLOC * (c + 1)]),
                "rv": np.ascontiguousarray(rvec[BLOC * c : BLOC * (c + 1)]),
                "w1": W1,
                "bias1": b1,
                "w2": W2,
                "bias2": b2,
                "emb": emb,
                "cwp": cwpa,
                "cb": conv_b,
                "sel": selm,
                "identin": identm,
            }
        )
    return in_maps


def kernel(**inputs):
    from concourse.bass_utils import run_bass_kernel_spmd

    nc = _get_nc()
    in_maps = make_in_maps(inputs)
    res = run_bass_kernel_spmd(nc, in_maps, core_ids=list(range(NCORES)))
    return np.concatenate([r["out"] for r in res.results], axis=0)



# revision 6
# speedup vs baseline: 1.4448x; 1.4448x over previous
"""MoE routing layer on 8 Trainium2 NeuronCores (data-parallel over batch).

Per core (4 samples):
  routing MLP -> cosine sim vs embeddings -> softmax weights wf[4,10]
  w_eff[b] = sum_n wf[b,n] * conv_w[n]   (conv is linear in the weights)
  out[b] = conv2d(x[b], w_eff[b]) + b_eff[b]

Conv runs in 64x64 PE tiling mode: 4 independent tiles = (2 samples) x
(2 chunk parities), each tile streaming its own 7-row output chunk
(9 shifted matmuls over the flat 58-wide grid), so the full 128x128
array is busy.  x / conv_w / w_eff / out are fp16 (fp32 accumulation in
PSUM); DMA moves ~4.9MB/core in 128-partition transfers.
"""
import sys

sys.path.insert(0, "/opt/trn_rl_repo")

import numpy as np

import concourse.bass as bass
import concourse.mybir as mybir
from concourse.tile import TileContext

F32 = mybir.dt.float32
F16 = mybir.dt.float16
BF16 = mybir.dt.bfloat16
AF = mybir.ActivationFunctionType
ALU = mybir.AluOpType
AX = mybir.AxisListType

NCORES = 8
BLOC = 4           # samples per core
CIN = 64
COUT = 64
H = W = 58
HW = H * W         # 3364
HWP = HW + 4       # padded flat length
OH = OW = 56
NB = 10            # experts
EDIM = 64
RSIZE = 512
HID = 128
NTAP = 9
CH_ROWS = 7        # output rows per chunk
NCH = 8            # chunks per sample (8*7 = 56)
NFREE = CH_ROWS * W  # 406 <= 512 (one PSUM bank)
TAP_OFF = [dy * W + dx for dy in range(3) for dx in range(3)]
SLOT_GROUPS = [(0, 5), (5, NTAP)]  # pair-0 weff pipelining
NWARM_PRE = 6     # warmup matmuls interleaved with routing
NWARM_POST = 10    # warmup matmuls between routing and conv


def fix_sync_waits(nc, cap=2):
    """This walrus build allows at most `cap` sem waits per instruction.
    Splice same-engine NoOps carrying the excess waits right before any
    over-subscribed instruction (waits happen earlier => same semantics)."""
    uid = [0]
    for f in nc.m.functions:
        for blk in f.blocks:
            insts = blk.instructions  # live list
            i = 0
            while i < len(insts):
                inst = insts[i]
                si = inst.sync_info
                waits = list(si.on_wait) if si and si.on_wait else []
                icap = 1
                if len(waits) <= icap:
                    i += 1
                    continue
                keep, excess = waits[-icap:], waits[:-icap]
                for k in range(0, len(excess), icap):
                    nop = mybir.InstNoOp(
                        name=f"{inst.name}-wsplit{uid[0]}", ins=[], outs=[]
                    )
                    uid[0] += 1
                    nop.engine = inst.engine
                    nop.sync_info = mybir.SyncInfo(
                        on_wait=excess[k : k + icap], on_update=[]
                    )
                    nc.register_instruction(nop, overwrite=True)
                    insts.insert(i, nop)
                    i += 1
                inst.sync_info = mybir.SyncInfo(
                    on_wait=keep,
                    on_update=list(si.on_update) if si and si.on_update else [],
                )
                i += 1


def build():
    nc = bass.Bass(num_swdge_queues=4)
    x16 = nc.dram_tensor("x16", [BLOC, CIN, HW], F16, kind="ExternalInput")
    rvT = nc.dram_tensor("rvT", [128, 4, BLOC], F32, kind="ExternalInput")
    w1 = nc.dram_tensor("w1", [128, 4, HID], F32, kind="ExternalInput")
    bias1 = nc.dram_tensor("bias1", [HID, 1], F32, kind="ExternalInput")
    w2 = nc.dram_tensor("w2", [HID, EDIM], F32, kind="ExternalInput")
    bias2 = nc.dram_tensor("bias2", [EDIM, 1], F32, kind="ExternalInput")
    emb = nc.dram_tensor("emb", [NB, EDIM], F32, kind="ExternalInput")
    cwp = nc.dram_tensor("cwp", [128, NB, NTAP, COUT], F16, kind="ExternalInput")
    cb2 = nc.dram_tensor("cb2", [NB, 128], F32, kind="ExternalInput")
    sel = nc.dram_tensor("sel", [2, BLOC, 128], F32, kind="ExternalInput")
    ident16 = nc.dram_tensor("ident16", [16, 16], F32, kind="ExternalInput")
    out16 = nc.dram_tensor(
        "out16", [BLOC, NCH, COUT, CH_ROWS, OW], F16, kind="ExternalOutput"
    )

    with TileContext(nc) as tc:
        with (
            tc.tile_pool(name="consts", bufs=1) as consts,
            tc.tile_pool(name="work", bufs=2) as work,
            tc.tile_pool(name="stage", bufs=4) as stage,
            tc.tile_pool(name="psr", bufs=2, space="PSUM") as psr,
            tc.tile_pool(name="psc", bufs=3, space="PSUM") as psc,
        ):
            # ---------- DMA: small consts + cwp on sync ring, x on scalar ----------
            identsb = consts.tile([16, 16], F32, tag="identsb")
            nc.sync.dma_start(out=identsb[:], in_=ident16[:])
            rvTsb = consts.tile([128, 4, BLOC], F32, tag="rvTsb")
            nc.sync.dma_start(out=rvTsb[:], in_=rvT[:])
            w1sb = consts.tile([128, 4, HID], F32, tag="w1sb")
            nc.sync.dma_start(out=w1sb[:], in_=w1[:])
            b1sb = consts.tile([HID, 1], F32, tag="b1sb")
            nc.sync.dma_start(out=b1sb[:], in_=bias1[:])
            w2sb = consts.tile([HID, EDIM], F32, tag="w2sb")
            nc.sync.dma_start(out=w2sb[:], in_=w2[:])
            b2sb = consts.tile([EDIM, 1], F32, tag="b2sb")
            nc.sync.dma_start(out=b2sb[:], in_=bias2[:])
            embsb = consts.tile([NB, EDIM], F32, tag="embsb")
            nc.sync.dma_start(out=embsb[:], in_=emb[:])
            cb2sb = consts.tile([NB, 128], F32, tag="cb2sb")
            nc.sync.dma_start(out=cb2sb[:], in_=cb2[:])
            selsb = consts.tile([BLOC, 2, 128], F32, tag="selsb")
            nc.sync.dma_start(out=selsb[:], in_=sel[:].rearrange("j b p -> b j p"))

            # conv weights, split by slot group so weff can start early
            cwpsb = consts.tile([128, NB, NTAP, COUT], F16, tag="cwpsb")
            for lo, hi in SLOT_GROUPS:
                nc.sync.dma_start(
                    out=cwpsb[:, :, lo:hi, :], in_=cwp[:, :, lo:hi, :]
                )

            # x: one 128-partition transfer per sample pair (all 16 SDMA engines)
            xt = consts.tile([128, 2, HWP], F16, tag="xt")
            nc.vector.memset(xt[:, :, HW:HWP], 0.0)
            for j in range(2):
                nc.scalar.dma_start(
                    out=xt[:, j, 0:HW],
                    in_=x16[2 * j : 2 * j + 2].rearrange("b c f -> (b c) f"),
                )

            ones64 = consts.tile([EDIM, 1], F32, tag="ones64")
            nc.vector.memset(ones64[:], 1.0)

            # ---------- PE warmup helper (keeps HAM clock-gate open) ----------
            warm_rhs = w1sb[:].rearrange("p c m -> p (c m)").bitcast(BF16)
            warm_lhs = warm_rhs[:, 0:128]

            def emit_warm(k):
                for _ in range(k):
                    wps = psc.tile([128, NFREE], F32, tag="pA", name="warmps")
                    nc.tensor.matmul(
                        wps[:, 0:384], warm_lhs, warm_rhs[:, 0:384],
                        start=True, stop=True,
                    )

            # ---------- routing MLP ----------
            h1ps = psr.tile([HID, BLOC], F32, tag="small")
            for c in range(4):
                nc.tensor.matmul(
                    h1ps[:], w1sb[:, c, :], rvTsb[:, c, :],
                    start=(c == 0), stop=(c == 3),
                )
            emit_warm(1)
            h1r = work.tile([HID, BLOC], F32, tag="h1r")
            nc.scalar.activation(
                out=h1r[:], in_=h1ps[:], func=AF.Relu, bias=b1sb[:], scale=1.0
            )
            rps = psr.tile([EDIM, BLOC], F32, tag="small")
            nc.tensor.matmul(rps[:], w2sb[:], h1r[:], start=True, stop=True)
            emit_warm(1)
            rsb = work.tile([EDIM, BLOC], F32, tag="rsb")
            nc.scalar.activation(
                out=rsb[:], in_=rps[:], func=AF.Identity, bias=b2sb[:], scale=1.0
            )

            # ---------- cosine similarity ----------
            rsq = work.tile([EDIM, BLOC], F32, tag="rsq")
            nc.vector.tensor_mul(rsq[:], rsb[:], rsb[:])
            nsq = psr.tile([BLOC, 1], F32, tag="small")
            nc.tensor.matmul(nsq[:], rsq[:], ones64[:], start=True, stop=True)
            rln = work.tile([BLOC, 1], F32, tag="rln")
            nc.scalar.activation(out=rln[:], in_=nsq[:], func=AF.Ln)
            rinv = work.tile([BLOC, 1], F32, tag="rinv")
            nc.scalar.activation(out=rinv[:], in_=rln[:], func=AF.Exp, scale=-0.5)

            esq = work.tile([NB, EDIM], F32, tag="esq")
            nc.vector.tensor_mul(esq[:], embsb[:], embsb[:])
            ensq = work.tile([NB, 1], F32, tag="ensq")
            nc.vector.tensor_reduce(ensq[:], esq[:], axis=AX.X, op=ALU.add)
            eln = work.tile([NB, 1], F32, tag="eln")
            nc.scalar.activation(out=eln[:], in_=ensq[:], func=AF.Ln)
            einv = work.tile([NB, 1], F32, tag="einv")
            nc.scalar.activation(out=einv[:], in_=eln[:], func=AF.Exp, scale=-0.5)
            embn = work.tile([NB, EDIM], F32, tag="embn")
            nc.vector.tensor_scalar_mul(out=embn[:], in0=embsb[:], scalar1=einv[:])
            embnT_ps = psr.tile([EDIM, NB], F32, tag="small")
            nc.tensor.transpose(embnT_ps[:], embn[:], identsb[0:NB, 0:NB])
            emit_warm(1)
            embnT = work.tile([EDIM, NB], F32, tag="embnT")
            nc.scalar.copy(out=embnT[:], in_=embnT_ps[:])

            simps = psr.tile([BLOC, NB], F32, tag="small")
            nc.tensor.matmul(simps[:], rsb[:], embnT[:], start=True, stop=True)
            emit_warm(1)
            sim = work.tile([BLOC, NB], F32, tag="sim")
            nc.vector.tensor_scalar_mul(out=sim[:], in0=simps[:], scalar1=rinv[:])

            # ---------- softmax ----------
            mx = work.tile([BLOC, 1], F32, tag="mx")
            nc.vector.tensor_reduce(mx[:], sim[:], axis=AX.X, op=ALU.max)
            negmx = work.tile([BLOC, 1], F32, tag="negmx")
            nc.vector.tensor_scalar_mul(out=negmx[:], in0=mx[:], scalar1=-1.0)
            ex = work.tile([BLOC, NB], F32, tag="ex")
            nc.scalar.activation(
                out=ex[:], in_=sim[:], func=AF.Exp, bias=negmx[:], scale=1.0
            )
            s = work.tile([BLOC, 1], F32, tag="s")
            nc.vector.tensor_reduce(s[:], ex[:], axis=AX.X, op=ALU.add)
            sinv = work.tile([BLOC, 1], F32, tag="sinv")
            nc.vector.reciprocal(sinv[:], s[:])
            wf = work.tile([BLOC, NB], F32, tag="wf")
            nc.vector.tensor_scalar_mul(out=wf[:], in0=ex[:], scalar1=sinv[:])

            # ---------- effective conv bias (per sample, dup'd across halves) ----------
            wfT_ps = psr.tile([NB, BLOC], F32, tag="small")
            nc.tensor.transpose(wfT_ps[:], wf[:], identsb[0:BLOC, 0:BLOC])
            wfT = work.tile([NB, BLOC], F32, tag="wfT")
            nc.scalar.copy(out=wfT[:], in_=wfT_ps[:])
            beff_ps = psr.tile([128, BLOC], F32, tag="small")
            nc.tensor.matmul(beff_ps[:], cb2sb[:], wfT[:], start=True, stop=True)
            beffSB = work.tile([128, BLOC], F32, tag="beffSB")
            nc.scalar.copy(out=beffSB[:], in_=beff_ps[:])

            # ---------- per-pair weight broadcast + w_eff (fp16 STT chains) ----------
            wfbcs = []
            for j in range(2):
                wfbc_ps = psr.tile([128, NB], F32, tag="small")
                nc.tensor.matmul(
                    wfbc_ps[:], selsb[:, j, :], wf[:], start=True, stop=True
                )
                wfbc = work.tile([128, NB], F32, tag=f"wfbc{j}")
                nc.scalar.copy(out=wfbc[:], in_=wfbc_ps[:])
                wfbcs.append(wfbc)

            emit_warm(NWARM_PRE)

            weffs = []
            for j in range(2):
                weff = work.tile([128, NTAP, COUT], F16, tag=f"weff{j}")
                groups = SLOT_GROUPS if j == 0 else [(0, NTAP)]
                for lo, hi in groups:
                    nc.vector.tensor_scalar_mul(
                        out=weff[:, lo:hi],
                        in0=cwpsb[:, 0, lo:hi, :],
                        scalar1=wfbcs[j][:, 0:1],
                    )
                    for n in range(1, NB):
                        nc.vector.scalar_tensor_tensor(
                            out=weff[:, lo:hi],
                            in0=cwpsb[:, n, lo:hi, :],
                            scalar=wfbcs[j][:, n : n + 1],
                            in1=weff[:, lo:hi],
                            op0=ALU.mult,
                            op1=ALU.add,
                        )
                weffs.append(weff)

            emit_warm(NWARM_POST)

            # ---------- conv: 64x64 tiling, 4 tiles = (2 samples)x(2 parities) ----------
            # group g covers chunks 4g..4g+3; bank r holds chunks 4g+2r (lo half)
            # and 4g+2r+1 (hi half) of one sample.
            for j in range(2):
                weff = weffs[j]
                for g in range(2):
                    pa = [
                        psc.tile([128, NFREE], F32, tag="pA", name=f"pa{j}{g}{r}")
                        for r in range(2)
                    ]
                    pb = [
                        psc.tile([128, NFREE], F32, tag="pB", name=f"pb{j}{g}{r}")
                        for r in range(2)
                    ]
                    for t in range(NTAP):
                        first = t == 0
                        last = t == NTAP - 1
                        for r in range(2):
                            c0 = 4 * g + 2 * r
                            offe = 7 * c0 * W + TAP_OFF[t]
                            offo = 7 * (c0 + 1) * W + TAP_OFF[t]
                            # sample A: tiles T0 (even chunk) / T2 (odd chunk)
                            nc.tensor.matmul(
                                pa[r][0:64],
                                weff[0:64, t, :],
                                xt[0:64, j, offe : offe + NFREE],
                                start=first, stop=last,
                            )
                            nc.tensor.matmul(
                                pa[r][64:128],
                                weff[0:64, t, :],
                                xt[0:64, j, offo : offo + NFREE],
                                start=first, stop=last,
                            )
                            # sample B: tiles T8 / T10
                            nc.tensor.matmul(
                                pb[r][0:64],
                                weff[64:128, t, :],
                                xt[64:128, j, offe : offe + NFREE],
                                start=first, stop=last,
                            )
                            nc.tensor.matmul(
                                pb[r][64:128],
                                weff[64:128, t, :],
                                xt[64:128, j, offo : offo + NFREE],
                                start=first, stop=last,
                            )
                    # evict group banks: bias add + fp16 convert, then DMA out
                    for r in range(2):
                        for s, bank in ((0, pa[r]), (1, pb[r])):
                            b = 2 * j + s
                            st = stage.tile([128, CH_ROWS, OW], F16, tag="st")
                            psv = bank[:].rearrange("p (h w) -> p h w", w=W)[
                                :, :, 0:OW
                            ]
                            nc.scalar.activation(
                                out=st[:],
                                in_=psv,
                                func=AF.Identity,
                                bias=beffSB[:, b : b + 1],
                                scale=1.0,
                            )
                            c0 = 4 * g + 2 * r
                            dst = out16[b, c0 : c0 + 2].rearrange(
                                "g2 c h w -> (g2 c) h w"
                            )
                            eng = nc.sync if (r + s) % 2 == 0 else nc.scalar
                            eng.dma_start(out=dst, in_=st[:])

    fix_sync_waits(nc)
    return nc


_NC = None


def _get_nc():
    global _NC
    if _NC is None:
        _NC = build()
    return _NC


def make_in_maps(inputs):
    x = np.asarray(inputs["x"], dtype=np.float32)
    rvec = np.asarray(inputs["routing_vector"], dtype=np.float32)
    W1 = np.asarray(inputs["W1"], dtype=np.float32)
    b1 = np.asarray(inputs["b1"], dtype=np.float32).reshape(HID, 1)
    b2 = np.asarray(inputs["b2"], dtype=np.float32).reshape(EDIM, 1)
    W2 = np.ascontiguousarray(np.asarray(inputs["W2"], dtype=np.float32))
    emb = np.ascontiguousarray(np.asarray(inputs["emb"], dtype=np.float32))
    conv_w = np.asarray(inputs["conv_w"], dtype=np.float32)
    conv_b = np.asarray(inputs["conv_b"], dtype=np.float32)

    x16 = np.ascontiguousarray(
        x.reshape(32, CIN, HW).astype(np.float16)
    )
    # conv_w[n, co, ci, ky, kx] -> [ci, n, (ky kx), co], dup'd over halves
    cwp1 = conv_w.transpose(2, 0, 3, 4, 1).reshape(CIN, NB, NTAP, COUT)
    cwp_h = np.ascontiguousarray(
        np.concatenate([cwp1, cwp1], axis=0).astype(np.float16)
    )
    w1_h = np.ascontiguousarray(W1.reshape(4, 128, HID).transpose(1, 0, 2))
    cb2_h = np.ascontiguousarray(np.concatenate([conv_b, conv_b], axis=1))
    selm = np.zeros((2, BLOC, 128), np.float32)
    for j in range(2):
        selm[j, 2 * j, 0:64] = 1.0
        selm[j, 2 * j + 1, 64:128] = 1.0
    identm = np.eye(16, dtype=np.float32)

    in_maps = []
    for c in range(NCORES):
        rvc = rvec[BLOC * c : BLOC * (c + 1)]
        rvT_h = np.ascontiguousarray(
            rvc.reshape(BLOC, 4, 128).transpose(2, 1, 0)
        )
        in_maps.append(
            {
                "x16": np.ascontiguousarray(x16[BLOC * c : BLOC * (c + 1)]),
                "rvT": rvT_h,
                "w1": w1_h,
                "bias1": b1,
                "w2": W2,
                "bias2": b2,
                "emb": emb,
                "cwp": cwp_h,
                "cb2": cb2_h,
                "sel": selm,
                "ident16": identm,
            }
        )
    return in_maps


def kernel(**inputs):
    from concourse.bass_utils import run_bass_kernel_spmd

    nc = _get_nc()
    in_maps = make_in_maps(inputs)
    res = run_bass_kernel_spmd(nc, in_maps, core_ids=list(range(NCORES)))
    outs = []
    for r in res.results:
        o = np.asarray(r["out16"]).astype(np.float32)
        o = o.transpose(0, 2, 1, 3, 4).reshape(BLOC, COUT, OH, OW)
        outs.append(o)
    return np.concatenate(outs, axis=0)
